# revision 13
# baseline (speedup 1.0000x reference)
"""Trainium2 Bass kernel for LGAttention (global MHA + windowed local MHA).

Sharding: one attention head per NeuronCore (8 heads, 8 cores), SPMD.
v3: global S as a concurrent row-tile pair (rows 0:48 / 64:112 of the PE
array, M=128 k-tokens each, two k-blocks in flight); PV split into k-halves
accumulating into two banks (P=top halves, Q=bottom halves); exp alternates
between the scalar engine (exact) and the vector engine (two-term
phase-shifted Schraudolph bf16 bit-trick, max rel err 1.2%); local window
attention uses 4-way 64x64 array tiling with exact exp; out-projection runs
as a final deep-pipelined phase with K=112 against [wp; 0; wp] so the local
branch's parity halves sum inside the matmul. Host divides by softmax
denominators, un-permutes windows, sums the 8 per-head partials, adds biases.
"""

import sys

sys.path.insert(0, "/opt/trn_rl_repo")

import numpy as np
import ml_dtypes

import concourse.bass as bass
import concourse.mybir as mybir
import concourse.tile as tile
from concourse import bacc, bass_utils

BF16 = mybir.dt.bfloat16
F32 = mybir.dt.float32
I16 = mybir.dt.int16

B, N, C = 2, 3136, 384
H, HD, WS = 8, 48, 7
NT = B * N            # 6272 tokens total
WT = WS * WS          # 49 tokens per window
QB = 448              # q-tile (free dim) for global attention
VS = 49               # v_aug column stride: 48 v + 1 ones (denominator row)
SCALE = float(HD) ** -0.5
# two-term Schraudolph bf16 exp: exp(x) ~ bitcast(i16(A*x+B1)) + bitcast(i16(A*x+B2))
SCH_A = 128.0 * 1.4426950408889634
SCH_B1 = 16149.25
SCH_B2 = 16088.75
EXP = mybir.ActivationFunctionType.Exp
MUL = mybir.AluOpType.mult
ADD = mybir.AluOpType.add
ACT_JP = (0, 2, 4, 6, 8, 10, 12)   # kb-pairs whose exp runs on the scalar engine
NCHUNK = 4                          # input DMA chunking


def build_program():
    nc = bacc.Bacc(
        "TRN2",
        target_bir_lowering=False,
        debug=False,
        enable_asserts=False,
        num_devices=8,
    )

    din = {}
    for name, shape in [
        ("xT", (C, NT)), ("winT", (C, NT)),
        ("gwqk", (C, 112)), ("gwv", (C, HD)), ("gwp", (112, C)),
        ("lwqk", (C, 112)), ("lwv", (C, HD)), ("lwp", (112, C)),
    ]:
        din[name] = nc.dram_tensor(name, list(shape), BF16, kind="ExternalInput").ap()

    dout = {}
    for name, shape, dt in [
        ("g_out", (NT, C), BF16), ("l_out", (NT, C), BF16),
        ("g_den", (2, NT), BF16), ("l_den", (2, NT), BF16),
    ]:
        dout[name] = nc.dram_tensor(name, list(shape), dt, kind="ExternalOutput").ap()

    with tile.TileContext(nc) as tc:
        _emit(tc, nc, din, dout)

    nc.compile()
    return nc


def _emit(tc, nc, din, dout):
    from contextlib import ExitStack

    ctx = ExitStack()
    with ctx:
        persist = ctx.enter_context(tc.tile_pool(name="persist", bufs=1))
        psum = ctx.enter_context(tc.tile_pool(name="psum", bufs=2, space="PSUM"))
        work = ctx.enter_context(tc.tile_pool(name="work", bufs=3))

        # ---- weights first (small), then chunked bulk loads ----
        gwqk = persist.tile([128, 3 * 112], BF16, name="gwqk")
        lwqk = persist.tile([128, 3 * 112], BF16, name="lwqk")
        gwv = persist.tile([128, 3 * 48], BF16, name="gwv")
        lwv = persist.tile([128, 3 * 48], BF16, name="lwv")
        for c in range(3):
            nc.sync.dma_start(gwqk[:, c * 112:(c + 1) * 112], din["gwqk"][c * 128:(c + 1) * 128, :])
            nc.sync.dma_start(lwqk[:, c * 112:(c + 1) * 112], din["lwqk"][c * 128:(c + 1) * 128, :])
            nc.sync.dma_start(gwv[:, c * 48:(c + 1) * 48], din["gwv"][c * 128:(c + 1) * 128, :])
            nc.sync.dma_start(lwv[:, c * 48:(c + 1) * 48], din["lwv"][c * 128:(c + 1) * 128, :])
        gwp = persist.tile([112, C], BF16, name="gwp")
        lwp = persist.tile([112, C], BF16, name="lwp")
        nc.sync.dma_start(gwp[:, :], din["gwp"][:, :])
        nc.sync.dma_start(lwp[:, :], din["lwp"][:, :])

        xt = [persist.tile([128, NT], BF16, name=f"xt{c}") for c in range(3)]
        wt = [persist.tile([128, NT], BF16, name=f"wt{c}") for c in range(3)]
        CH = NT // NCHUNK
        for ch in range(NCHUNK):
            for c in range(3):
                nc.sync.dma_start(xt[c][:, ch * CH:(ch + 1) * CH],
                                  din["xT"][c * 128:(c + 1) * 128, ch * CH:(ch + 1) * CH])
        for ch in range(NCHUNK):
            for c in range(3):
                nc.sync.dma_start(wt[c][:, ch * CH:(ch + 1) * CH],
                                  din["winT"][c * 128:(c + 1) * 128, ch * CH:(ch + 1) * CH])

        # ---- persistent intermediates (q/k lo rows 0:48, hi rows 64:112) ----
        g_qT = persist.tile([128, NT], BF16, name="g_qT")
        g_kT = persist.tile([128, NT], BF16, name="g_kT")
        l_qT = persist.tile([128, NT], BF16, name="l_qT")
        l_kT = persist.tile([128, NT], BF16, name="l_kT")
        g_vaug = persist.tile([128, 50 * VS], BF16, name="g_vaug")
        l_vaug = persist.tile([128, 64 * VS], BF16, name="l_vaug")
        # out^T: global rows 0:48 (+den 48); local parity halves rows 0:48/64:112
        # (+dens 48/112). Out-projection contracts K=112 against [wp; 0; wp].
        g_outT = persist.tile([128, NT], BF16, name="g_outT")
        l_outT = persist.tile([128, NT], BF16, name="l_outT")
        nc.vector.memset(l_outT[:, :], 0.0)
        nc.vector.memset(g_outT[:, :], 0.0)

        # ones columns (softmax denominator comes out of the PV matmul, row 48)
        nc.vector.memset(g_vaug[:, :].rearrange("p (b k) -> p b k", k=VS)[:, :, 48:VS], 1.0)
        nc.vector.memset(l_vaug[:, :].rearrange("p (b k) -> p b k", k=VS)[:, :, 48:VS], 1.0)

        # ---- projections (x-consumers first, then win-consumers) ----
        def qk_proj(src, qT, kT, wqk):
            for qb in range(14):
                t0 = qb * QB
                ps = psum.tile([112, QB], F32, name="pqk", tag="psO", bufs=4)
                for c in range(3):
                    nc.tensor.matmul(ps[:, :], wqk[:, c * 112:(c + 1) * 112],
                                     src[c][:, t0:t0 + QB], start=(c == 0), stop=(c == 2))
                nc.scalar.copy(qT[0:48, t0:t0 + QB], ps[0:48, :])
                nc.vector.tensor_copy(kT[0:48, t0:t0 + QB], ps[64:112, :])
            for t in (qT, kT):
                nc.sync.dma_start(t[64:112, :], t[0:48, :])

        qk_proj(xt, g_qT, g_kT, gwqk)
        # global v projection (token-major), fills g_vaug cols 0:48
        for b in range(2):
            for j in range(25):
                sz = 128 if j < 24 else 64
                t0 = b * N + j * 128
                bl = b * 25 + j
                ps = psum.tile([128, HD], F32, name="pv", tag="psO", bufs=4)
                for c in range(3):
                    nc.tensor.matmul(ps[0:sz, :], xt[c][:, t0:t0 + sz],
                                     gwv[:, c * 48:(c + 1) * 48], start=(c == 0), stop=(c == 2))
                nc.vector.tensor_copy(g_vaug[0:sz, bl * VS:bl * VS + 48], ps[0:sz, :])

        qk_proj(wt, l_qT, l_kT, lwqk)
        # local v projection: col-tiled window pairs, 8 windows per psum
        for g8 in range(16):
            ps = psum.tile([128, 8 * 48], F32, name="pvl", tag="psO", bufs=4)
            for wi in range(8):
                w = g8 * 8 + wi
                t0 = w * WT
                r0 = 0 if w % 2 == 0 else 64
                for c in range(3):
                    nc.tensor.matmul(ps[r0:r0 + WT, wi * 48:(wi + 1) * 48],
                                     wt[c][:, t0:t0 + WT],
                                     lwv[:, c * 48:(c + 1) * 48], start=(c == 0), stop=(c == 2))
            dst_lo = l_vaug[0:WT, :].rearrange("p (w k) -> p w k", k=VS)[:, g8 * 4:(g8 + 1) * 4, 0:48]
            src_lo = ps[0:WT, :].rearrange("p (w k) -> p w k", k=48)[:, 0:8:2, :]
            dst_hi = l_vaug[64:64 + WT, :].rearrange("p (w k) -> p w k", k=VS)[:, g8 * 4:(g8 + 1) * 4, 0:48]
            src_hi = ps[64:64 + WT, :].rearrange("p (w k) -> p w k", k=48)[:, 1:8:2, :]
            nc.scalar.copy(dst_lo, src_lo)
            nc.vector.tensor_copy(dst_hi, src_hi)

        # ---- local attention: 8 iterations of 16 windows (2 S banks, exact exp) ----
        for it in range(8):
            psA = psum.tile([128, 8 * WT], F32, name="pSlA", tag="pS", bufs=2)
            psB = psum.tile([128, 8 * WT], F32, name="pSlB", tag="pS", bufs=2)
            for wi in range(16):
                w = it * 16 + wi
                t0 = w * WT
                bank = psA if wi < 8 else psB
                r0, r1 = (0, 48) if wi < 8 else (64, 112)
                orow = 0 if wi % 2 == 0 else 64
                col = (wi % 8) * WT
                nc.tensor.matmul(bank[orow:orow + WT, col:col + WT],
                                 l_kT[r0:r1, t0:t0 + WT], l_qT[r0:r1, t0:t0 + WT],
                                 start=True, stop=True)
            exA = work.tile([128, 8 * WT], BF16, name="expSlA", tag="exl", bufs=3)
            exB = work.tile([128, 8 * WT], BF16, name="expSlB", tag="exl2", bufs=3)

            def _wv(t, r0, par):
                return t[r0:r0 + VS, :].rearrange("p (w k) -> p w k", k=WT)[:, par:8:2, :]

            nc.scalar.activation(_wv(exA, 0, 0), _wv(psA, 0, 0), EXP, scale=SCALE)
            nc.scalar.activation(_wv(exA, 64, 1), _wv(psA, 64, 1), EXP, scale=SCALE)
            nc.scalar.activation(_wv(exB, 0, 0), _wv(psB, 0, 0), EXP, scale=SCALE)
            nc.scalar.activation(_wv(exB, 64, 1), _wv(psB, 64, 1), EXP, scale=SCALE)
            poA = psum.tile([128, 8 * WT], F32, name="poutlA", tag="psO", bufs=4)
            poB = psum.tile([128, 8 * WT], F32, name="poutlB", tag="psO", bufs=4)
            for wi in range(16):
                w = it * 16 + wi
                po = poA if wi < 8 else poB
                ex = exA if wi < 8 else exB
                col = (wi % 8) * WT
                vrow = 0 if w % 2 == 0 else 64
                nc.tensor.matmul(po[vrow:vrow + VS, col:col + WT],
                                 l_vaug[vrow:vrow + WT, (w // 2) * VS:(w // 2) * VS + VS],
                                 ex[vrow:vrow + WT, col:col + WT],
                                 start=True, stop=True)
            # evacuate per parity half (even windows rows 0:49, odd rows 64:113)
            w0 = it * 16
            for po, base in ((poA, 0), (poB, 8)):
                wb = w0 + base
                for par, vrow, eng in ((0, 0, "s"), (1, 64, "v")):
                    src_o = po[vrow:vrow + VS, :].rearrange("p (w k) -> p w k", k=WT)[:, par:8:2, :]
                    dst_o = l_outT[vrow:vrow + VS, wb * WT:(wb + 8) * WT].rearrange(
                        "p (w k) -> p w k", k=WT)[:, par:8:2, :]
                    if eng == "s":
                        nc.scalar.copy(dst_o, src_o)
                    else:
                        nc.vector.tensor_copy(dst_o, src_o)
            nc.sync.dma_start(dout["l_den"][0:1, w0 * WT:(w0 + 16) * WT],
                              l_outT[48:49, w0 * WT:(w0 + 16) * WT])
            nc.sync.dma_start(dout["l_den"][1:2, w0 * WT:(w0 + 16) * WT],
                              l_outT[112:113, w0 * WT:(w0 + 16) * WT])

        # ---- global attention: kb pair per iteration as a concurrent row-tile
        # pair (M=128); PV accumulates k-top halves into P and k-bottom halves
        # into Q; exp alternates scalar/vector engines; PV runs one kb-pair
        # behind S so the PE never waits on exp ----
        for b in range(2):
            for s in range(7):
                q0 = b * N + s * QB
                psP = psum.tile([128, QB], F32, name="psP", tag="psO", bufs=4)
                psQ = psum.tile([128, QB], F32, name="psQ", tag="psO", bufs=4)
                exs = [None] * 13
                for jp in range(14):
                    if jp < 13:
                        j0, j1 = 2 * jp, 2 * jp + 1
                        k0 = b * N + j0 * 128
                        k1 = b * N + j1 * 128
                        sz0 = 128 if j0 < 24 else 64
                        have1 = j1 < 25
                        # S(j0) at bank 0 cols 0:448, S(j1) at bank 1 cols 512:960
                        ps2 = psum.tile([128, 1024], F32, name="pS2", tag="pS", bufs=2)
                        nc.tensor.matmul(ps2[0:sz0, 0:QB], g_kT[0:48, k0:k0 + sz0],
                                         g_qT[0:48, q0:q0 + QB], start=True, stop=True)
                        if have1:
                            nc.tensor.matmul(ps2[0:128, 512:512 + QB],
                                             g_kT[64:112, k1:k1 + 128],
                                             g_qT[64:112, q0:q0 + QB], start=True, stop=True)
                        nu = 2 if have1 else 1
                        ps_v = (ps2[0:128, :].rearrange("p (u k) -> p u k", k=512)[:, 0:nu, 0:QB]
                                if nu == 2 else ps2[0:sz0, 0:QB])
                        if jp in ACT_JP:
                            ex = work.tile([128, 2 * QB], BF16, name="expA", tag="exA", bufs=3)
                            ex_v = (ex[0:128, :].rearrange("p (u k) -> p u k", k=QB)[:, 0:nu, :]
                                    if nu == 2 else ex[0:sz0, 0:QB])
                            nc.scalar.activation(ex_v, ps_v, EXP, scale=SCALE)
                            exs[jp] = ex
                        else:
                            e1 = work.tile([128, 2 * QB], I16, name="exi1", tag="exi1", bufs=3)
                            e2 = work.tile([128, 2 * QB], I16, name="exi2", tag="exi2", bufs=3)
                            ex = work.tile([128, 2 * QB], BF16, name="expB", tag="exB", bufs=3)
                            for e, bconst in ((e1, SCH_B1), (e2, SCH_B2)):
                                e_v = (e[0:128, :].rearrange("p (u k) -> p u k", k=QB)[:, 0:nu, :]
                                       if nu == 2 else e[0:sz0, 0:QB])
                                nc.vector.tensor_scalar(e_v, ps_v,
                                                        SCALE * SCH_A, bconst, MUL, ADD)
                            rows = 128 if nu == 2 else sz0
                            wid = 2 * QB if nu == 2 else QB
                            nc.vector.tensor_tensor(ex[0:rows, 0:wid],
                                                    e1[0:rows, 0:wid].bitcast(BF16),
                                                    e2[0:rows, 0:wid].bitcast(BF16), ADD)
                            exs[jp] = ex
                    if jp >= 1:
                        jj = jp - 1
                        ex = exs[jj]
                        j0, j1 = 2 * jj, 2 * jj + 1
                        blA = b * 25 + j0
                        blB = b * 25 + j1
                        # kb j0: top half -> P, bottom half -> Q
                        nc.tensor.matmul(psP[0:VS, :], g_vaug[0:64, blA * VS:blA * VS + VS],
                                         ex[0:64, 0:QB], start=(jj == 0), stop=(jj == 12))
                        if j0 < 24:
                            nc.tensor.matmul(psQ[0:VS, :],
                                             g_vaug[64:128, blA * VS:blA * VS + VS],
                                             ex[64:128, 0:QB], start=(jj == 0), stop=False)
                        if j1 < 25:
                            nc.tensor.matmul(psP[0:VS, :],
                                             g_vaug[0:64, blB * VS:blB * VS + VS],
                                             ex[0:64, QB:2 * QB], start=False, stop=False)
                            nc.tensor.matmul(psQ[0:VS, :],
                                             g_vaug[64:128, blB * VS:blB * VS + VS],
                                             ex[64:128, QB:2 * QB], start=False,
                                             stop=(jj == 11))
                # evacuate: P+Q -> outT rows 0:48, den row 48
                t1 = work.tile([128, QB], F32, name="t1", tag="t1", bufs=2)
                nc.scalar.copy(t1[0:VS, :], psQ[0:VS, :])
                nc.vector.tensor_tensor(g_outT[0:VS, q0:q0 + QB],
                                        psP[0:VS, :], t1[0:VS, :], ADD)
                nc.sync.dma_start(dout["g_den"][0:1, q0:q0 + QB],
                                  g_outT[48:49, q0:q0 + QB])
        # g_den row 1 stays zero (memset) -- host sums both rows
        nc.sync.dma_start(dout["g_den"][1:2, :], g_outT[112:113, :])

        # ---- final out-projection phase (deep ring, K=112 with [wp; 0; wp]) ----
        for blk in range(56):
            t0 = blk * 112
            for outT, wp, dst, eng in ((g_outT, gwp, dout["g_out"], "v"),
                                       (l_outT, lwp, dout["l_out"],
                                        "s" if blk % 3 == 2 else "v")):
                pp = psum.tile([112, C], F32, name="pp", tag="psO", bufs=4)
                nc.tensor.matmul(pp[:, :], outT[0:112, t0:t0 + 112], wp[:, :],
                                 start=True, stop=True)
                sp = work.tile([112, C], BF16, name="sproj", tag="sproj", bufs=6)
                if eng == "v":
                    nc.vector.tensor_copy(sp[:, :], pp[:, :])
                else:
                    nc.scalar.copy(sp[:, :], pp[:, :])
                nc.sync.dma_start(dst[t0:t0 + 112, :], sp[:, :])


def _host_prep(x, g_qkv_w, g_proj_w, l_qkv_w, l_proj_w):
    bf = ml_dtypes.bfloat16
    xf = np.asarray(x, np.float32).reshape(NT, C)
    xT = np.ascontiguousarray(xf.T).astype(bf)
    x4 = np.asarray(x, np.float32).reshape(B, 56, 56, C)
    win = x4.reshape(B, 8, WS, 8, WS, C).transpose(0, 1, 3, 5, 2, 4)
    win = win.reshape(B, 8, 8, WS, WS, C).transpose(0, 1, 2, 4, 3, 5).reshape(NT, C)
    winT = np.ascontiguousarray(win.T).astype(bf)

    in_maps = []
    for h in range(8):
        m = {"xT": xT, "winT": winT}
        for pre, qkv_w, proj_w in (("g", g_qkv_w, g_proj_w), ("l", l_qkv_w, l_proj_w)):
            qw = np.asarray(qkv_w[:, h * HD:(h + 1) * HD], np.float32)
            kw = np.asarray(qkv_w[:, C + h * HD:C + (h + 1) * HD], np.float32)
            vw = np.asarray(qkv_w[:, 2 * C + h * HD:2 * C + (h + 1) * HD], np.float32)
            wqk = np.zeros((C, 112), np.float32)
            wqk[:, 0:48] = qw
            wqk[:, 64:112] = kw
            m[pre + "wqk"] = wqk.astype(bf)
            m[pre + "wv"] = np.ascontiguousarray(vw).astype(bf)
            wph = np.asarray(proj_w, np.float32)[h * HD:(h + 1) * HD, :]
            wp2 = np.zeros((112, C), np.float32)
            wp2[0:48] = wph
            wp2[64:112] = wph
            m[pre + "wp"] = wp2.astype(bf)
        in_maps.append(m)
    return in_maps


_NC_CACHE = None


def kernel(x, g_qkv_w, g_proj_w, g_proj_b, l_qkv_w, l_proj_w, l_proj_b):
    global _NC_CACHE
    if _NC_CACHE is None:
        _NC_CACHE = build_program()
    nc = _NC_CACHE

    in_maps = _host_prep(x, g_qkv_w, g_proj_w, l_qkv_w, l_proj_w)
    res = bass_utils.run_bass_kernel_spmd(nc, in_maps, core_ids=list(range(8)))

    acc = np.zeros((NT, C), np.float32)
    l_acc = np.zeros((NT, C), np.float32)
    for h in range(8):
        r = res.results[h]
        gden = np.asarray(r["g_den"], np.float32).sum(0).reshape(NT, 1)
        lden = np.asarray(r["l_den"], np.float32).sum(0).reshape(NT, 1)
        acc += np.asarray(r["g_out"], np.float32) / gden
        l_acc += np.asarray(r["l_out"], np.float32) / lden
    l_tok = l_acc.reshape(B, 8, 8, WS, WS, C).transpose(0, 1, 3, 2, 4, 5).reshape(NT, C)
    out = acc + l_tok + np.asarray(g_proj_b, np.float32) + np.asarray(l_proj_b, np.float32)
    return out.reshape(B, N, C).astype(np.float32)


# revision 14
# speedup vs baseline: 1.0677x; 1.0677x over previous
"""Trainium2 Bass kernel for LGAttention (global MHA + windowed local MHA).

Sharding: one attention head per NeuronCore (8 heads, 8 cores), SPMD.
v3: global S as a concurrent row-tile pair (rows 0:48 / 64:112 of the PE
array, M=128 k-tokens each, two k-blocks in flight); PV split into k-halves
accumulating into two banks (P=top halves, Q=bottom halves); exp alternates
between the scalar engine (exact) and the vector engine (two-term
phase-shifted Schraudolph bf16 bit-trick, max rel err 1.2%); local window
attention uses 4-way 64x64 array tiling with exact exp; out-projection runs
as a final deep-pipelined phase with K=112 against [wp; 0; wp] so the local
branch's parity halves sum inside the matmul. Host divides by softmax
denominators, un-permutes windows, sums the 8 per-head partials, adds biases.
"""

import sys

sys.path.insert(0, "/opt/trn_rl_repo")

import numpy as np
import ml_dtypes

import concourse.bass as bass
import concourse.mybir as mybir
import concourse.tile as tile
from concourse import bacc, bass_utils

BF16 = mybir.dt.bfloat16
F32 = mybir.dt.float32
I16 = mybir.dt.int16

B, N, C = 2, 3136, 384
H, HD, WS = 8, 48, 7
NT = B * N            # 6272 tokens total
WT = WS * WS          # 49 tokens per window
QB = 448              # q-tile (free dim) for global attention
VS = 49               # v_aug column stride: 48 v + 1 ones (denominator row)
SCALE = float(HD) ** -0.5
# two-term Schraudolph bf16 exp: exp(x) ~ bitcast(i16(A*x+B1)) + bitcast(i16(A*x+B2))
SCH_A = 128.0 * 1.4426950408889634
SCH_B1 = 16149.25
SCH_B2 = 16088.75
EXP = mybir.ActivationFunctionType.Exp
MUL = mybir.AluOpType.mult
ADD = mybir.AluOpType.add
ACT_JP = (0, 1, 2, 3, 4, 6, 7, 9, 10, 12)   # kb-pairs with exact exp (scalar engine)
NCHUNK = 4                          # input DMA chunking


def build_program():
    nc = bacc.Bacc(
        "TRN2",
        target_bir_lowering=False,
        debug=False,
        enable_asserts=False,
        num_devices=8,
    )

    din = {}
    for name, shape in [
        ("xT", (C, NT)), ("winT", (C, NT)),
        ("gwqk", (C, 112)), ("gwv", (C, HD)), ("gwp", (112, C)),
        ("lwqk", (C, 112)), ("lwv", (C, HD)), ("lwp", (112, C)),
    ]:
        din[name] = nc.dram_tensor(name, list(shape), BF16, kind="ExternalInput").ap()

    dout = {}
    for name, shape, dt in [
        ("g_out", (NT, C), BF16), ("l_out", (NT, C), BF16),
        ("g_den", (2, NT), BF16), ("l_den", (2, NT), BF16),
    ]:
        dout[name] = nc.dram_tensor(name, list(shape), dt, kind="ExternalOutput").ap()

    with tile.TileContext(nc) as tc:
        _emit(tc, nc, din, dout)

    nc.compile()
    return nc


def _emit(tc, nc, din, dout):
    from contextlib import ExitStack

    ctx = ExitStack()
    with ctx:
        persist = ctx.enter_context(tc.tile_pool(name="persist", bufs=1))
        psum = ctx.enter_context(tc.tile_pool(name="psum", bufs=2, space="PSUM"))
        work = ctx.enter_context(tc.tile_pool(name="work", bufs=3))

        # ---- weights first (small), then chunked bulk loads ----
        gwqk = persist.tile([128, 3 * 112], BF16, name="gwqk")
        lwqk = persist.tile([128, 3 * 112], BF16, name="lwqk")
        gwv = persist.tile([128, 3 * 48], BF16, name="gwv")
        lwv = persist.tile([128, 3 * 48], BF16, name="lwv")
        for c in range(3):
            nc.sync.dma_start(gwqk[:, c * 112:(c + 1) * 112], din["gwqk"][c * 128:(c + 1) * 128, :])
            nc.sync.dma_start(lwqk[:, c * 112:(c + 1) * 112], din["lwqk"][c * 128:(c + 1) * 128, :])
            nc.sync.dma_start(gwv[:, c * 48:(c + 1) * 48], din["gwv"][c * 128:(c + 1) * 128, :])
            nc.sync.dma_start(lwv[:, c * 48:(c + 1) * 48], din["lwv"][c * 128:(c + 1) * 128, :])
        gwp = persist.tile([112, C], BF16, name="gwp")
        lwp = persist.tile([112, C], BF16, name="lwp")
        nc.sync.dma_start(gwp[:, :], din["gwp"][:, :])
        nc.sync.dma_start(lwp[:, :], din["lwp"][:, :])

        xt = [persist.tile([128, NT], BF16, name=f"xt{c}") for c in range(3)]
        wt = [persist.tile([128, NT], BF16, name=f"wt{c}") for c in range(3)]
        CH = NT // NCHUNK
        for ch in range(NCHUNK):
            for c in range(3):
                nc.sync.dma_start(xt[c][:, ch * CH:(ch + 1) * CH],
                                  din["xT"][c * 128:(c + 1) * 128, ch * CH:(ch + 1) * CH])
        for ch in range(NCHUNK):
            for c in range(3):
                nc.sync.dma_start(wt[c][:, ch * CH:(ch + 1) * CH],
                                  din["winT"][c * 128:(c + 1) * 128, ch * CH:(ch + 1) * CH])

        # ---- persistent intermediates (q/k lo rows 0:48, hi rows 64:112) ----
        g_qT = persist.tile([128, NT], BF16, name="g_qT")
        g_kT = persist.tile([128, NT], BF16, name="g_kT")
        l_qT = persist.tile([128, NT], BF16, name="l_qT")
        l_kT = persist.tile([128, NT], BF16, name="l_kT")
        g_vaug = persist.tile([128, 50 * VS], BF16, name="g_vaug")
        l_vaug = persist.tile([128, 64 * VS], BF16, name="l_vaug")
        # out^T: global rows 0:48 (+den 48); local parity halves rows 0:48/64:112
        # (+dens 48/112). Out-projection contracts K=112 against [wp; 0; wp].
        g_outT = persist.tile([128, NT], BF16, name="g_outT")
        l_outT = persist.tile([128, NT], BF16, name="l_outT")
        nc.vector.memset(l_outT[:, :], 0.0)
        nc.vector.memset(g_outT[:, :], 0.0)

        # ones columns (softmax denominator comes out of the PV matmul, row 48)
        nc.vector.memset(g_vaug[:, :].rearrange("p (b k) -> p b k", k=VS)[:, :, 48:VS], 1.0)
        nc.vector.memset(l_vaug[:, :].rearrange("p (b k) -> p b k", k=VS)[:, :, 48:VS], 1.0)

        # ---- projections (x-consumers first, then win-consumers) ----
        def qk_proj(src, qT, kT, wqk):
            for qb in range(14):
                t0 = qb * QB
                ps = psum.tile([112, QB], F32, name="pqk", tag="psO", bufs=4)
                for c in range(3):
                    nc.tensor.matmul(ps[:, :], wqk[:, c * 112:(c + 1) * 112],
                                     src[c][:, t0:t0 + QB], start=(c == 0), stop=(c == 2))
                nc.scalar.copy(qT[0:48, t0:t0 + QB], ps[0:48, :])
                nc.vector.tensor_copy(kT[0:48, t0:t0 + QB], ps[64:112, :])
            for t in (qT, kT):
                nc.sync.dma_start(t[64:112, :], t[0:48, :])

        qk_proj(xt, g_qT, g_kT, gwqk)
        # global v projection (token-major), fills g_vaug cols 0:48
        for b in range(2):
            for j in range(25):
                sz = 128 if j < 24 else 64
                t0 = b * N + j * 128
                bl = b * 25 + j
                ps = psum.tile([128, HD], F32, name="pv", tag="psO", bufs=4)
                for c in range(3):
                    nc.tensor.matmul(ps[0:sz, :], xt[c][:, t0:t0 + sz],
                                     gwv[:, c * 48:(c + 1) * 48], start=(c == 0), stop=(c == 2))
                nc.vector.tensor_copy(g_vaug[0:sz, bl * VS:bl * VS + 48], ps[0:sz, :])

        qk_proj(wt, l_qT, l_kT, lwqk)
        # local v projection: col-tiled window pairs, 8 windows per psum
        for g8 in range(16):
            ps = psum.tile([128, 8 * 48], F32, name="pvl", tag="psO", bufs=4)
            for wi in range(8):
                w = g8 * 8 + wi
                t0 = w * WT
                r0 = 0 if w % 2 == 0 else 64
                for c in range(3):
                    nc.tensor.matmul(ps[r0:r0 + WT, wi * 48:(wi + 1) * 48],
                                     wt[c][:, t0:t0 + WT],
                                     lwv[:, c * 48:(c + 1) * 48], start=(c == 0), stop=(c == 2))
            dst_lo = l_vaug[0:WT, :].rearrange("p (w k) -> p w k", k=VS)[:, g8 * 4:(g8 + 1) * 4, 0:48]
            src_lo = ps[0:WT, :].rearrange("p (w k) -> p w k", k=48)[:, 0:8:2, :]
            dst_hi = l_vaug[64:64 + WT, :].rearrange("p (w k) -> p w k", k=VS)[:, g8 * 4:(g8 + 1) * 4, 0:48]
            src_hi = ps[64:64 + WT, :].rearrange("p (w k) -> p w k", k=48)[:, 1:8:2, :]
            nc.scalar.copy(dst_lo, src_lo)
            nc.vector.tensor_copy(dst_hi, src_hi)

        # ---- local attention: 8 iterations of 16 windows (2 S banks, exact exp) ----
        for it in range(8):
            psA = psum.tile([128, 8 * WT], F32, name="pSlA", tag="pS", bufs=2)
            psB = psum.tile([128, 8 * WT], F32, name="pSlB", tag="pS", bufs=2)
            for wi in range(16):
                w = it * 16 + wi
                t0 = w * WT
                bank = psA if wi < 8 else psB
                r0, r1 = (0, 48) if wi < 8 else (64, 112)
                orow = 0 if wi % 2 == 0 else 64
                col = (wi % 8) * WT
                nc.tensor.matmul(bank[orow:orow + WT, col:col + WT],
                                 l_kT[r0:r1, t0:t0 + WT], l_qT[r0:r1, t0:t0 + WT],
                                 start=True, stop=True)
            exA = work.tile([128, 8 * WT], BF16, name="expSlA", tag="exl", bufs=3)
            exB = work.tile([128, 8 * WT], BF16, name="expSlB", tag="exl2", bufs=3)

            def _wv(t, r0, par):
                return t[r0:r0 + VS, :].rearrange("p (w k) -> p w k", k=WT)[:, par:8:2, :]

            nc.scalar.activation(_wv(exA, 0, 0), _wv(psA, 0, 0), EXP, scale=SCALE)
            nc.scalar.activation(_wv(exA, 64, 1), _wv(psA, 64, 1), EXP, scale=SCALE)
            nc.scalar.activation(_wv(exB, 0, 0), _wv(psB, 0, 0), EXP, scale=SCALE)
            nc.scalar.activation(_wv(exB, 64, 1), _wv(psB, 64, 1), EXP, scale=SCALE)
            poA = psum.tile([128, 8 * WT], F32, name="poutlA", tag="psO", bufs=4)
            poB = psum.tile([128, 8 * WT], F32, name="poutlB", tag="psO", bufs=4)
            for wi in range(16):
                w = it * 16 + wi
                po = poA if wi < 8 else poB
                ex = exA if wi < 8 else exB
                col = (wi % 8) * WT
                vrow = 0 if w % 2 == 0 else 64
                nc.tensor.matmul(po[vrow:vrow + VS, col:col + WT],
                                 l_vaug[vrow:vrow + WT, (w // 2) * VS:(w // 2) * VS + VS],
                                 ex[vrow:vrow + WT, col:col + WT],
                                 start=True, stop=True)
            # evacuate per parity half (even windows rows 0:49, odd rows 64:113)
            w0 = it * 16
            for po, base in ((poA, 0), (poB, 8)):
                wb = w0 + base
                for par, vrow, eng in ((0, 0, "s"), (1, 64, "v")):
                    src_o = po[vrow:vrow + VS, :].rearrange("p (w k) -> p w k", k=WT)[:, par:8:2, :]
                    dst_o = l_outT[vrow:vrow + VS, wb * WT:(wb + 8) * WT].rearrange(
                        "p (w k) -> p w k", k=WT)[:, par:8:2, :]
                    if eng == "s":
                        nc.scalar.copy(dst_o, src_o)
                    else:
                        nc.vector.tensor_copy(dst_o, src_o)
            nc.sync.dma_start(dout["l_den"][0:1, w0 * WT:(w0 + 16) * WT],
                              l_outT[48:49, w0 * WT:(w0 + 16) * WT])
            nc.sync.dma_start(dout["l_den"][1:2, w0 * WT:(w0 + 16) * WT],
                              l_outT[112:113, w0 * WT:(w0 + 16) * WT])

        # ---- global attention: kb pair per iteration as a concurrent row-tile
        # pair (M=128); PV accumulates k-top halves into P and k-bottom halves
        # into Q; exp alternates scalar/vector engines; PV runs one kb-pair
        # behind S so the PE never waits on exp ----
        for b in range(2):
            for s in range(7):
                q0 = b * N + s * QB
                psW = psum.tile([128, QB], F32, name="psW", tag="psO", bufs=4)
                psX = psum.tile([128, QB], F32, name="psX", tag="psO", bufs=4)
                psY = psum.tile([128, QB], F32, name="psY", tag="psO", bufs=4)
                psZ = psum.tile([128, QB], F32, name="psZ", tag="psO", bufs=4)
                exs = [None] * 13
                for jp in range(14):
                    if jp < 13:
                        j0, j1 = 2 * jp, 2 * jp + 1
                        k0 = b * N + j0 * 128
                        k1 = b * N + j1 * 128
                        sz0 = 128 if j0 < 24 else 64
                        have1 = j1 < 25
                        # S(j0) at bank 0 cols 0:448, S(j1) at bank 1 cols 512:960
                        ps2 = psum.tile([128, 1024], F32, name="pS2", tag="pS", bufs=2)
                        nc.tensor.matmul(ps2[0:sz0, 0:QB], g_kT[0:48, k0:k0 + sz0],
                                         g_qT[0:48, q0:q0 + QB], start=True, stop=True)
                        if have1:
                            nc.tensor.matmul(ps2[0:128, 512:512 + QB],
                                             g_kT[64:112, k1:k1 + 128],
                                             g_qT[64:112, q0:q0 + QB], start=True, stop=True)
                        nu = 2 if have1 else 1
                        ps_v = (ps2[0:128, :].rearrange("p (u k) -> p u k", k=512)[:, 0:nu, 0:QB]
                                if nu == 2 else ps2[0:sz0, 0:QB])
                        if jp in ACT_JP:
                            ex = work.tile([128, 2 * QB], BF16, name="expA", tag="exA", bufs=3)
                            ex_v = (ex[0:128, :].rearrange("p (u k) -> p u k", k=QB)[:, 0:nu, :]
                                    if nu == 2 else ex[0:sz0, 0:QB])
                            nc.scalar.activation(ex_v, ps_v, EXP, scale=SCALE)
                            exs[jp] = ex
                        else:
                            e1 = work.tile([128, 2 * QB], I16, name="exi1", tag="exi1", bufs=3)
                            e2 = work.tile([128, 2 * QB], I16, name="exi2", tag="exi2", bufs=3)
                            ex = work.tile([128, 2 * QB], BF16, name="expB", tag="exB", bufs=3)
                            for e, bconst in ((e1, SCH_B1), (e2, SCH_B2)):
                                e_v = (e[0:128, :].rearrange("p (u k) -> p u k", k=QB)[:, 0:nu, :]
                                       if nu == 2 else e[0:sz0, 0:QB])
                                nc.vector.tensor_scalar(e_v, ps_v,
                                                        SCALE * SCH_A, bconst, MUL, ADD)
                            rows = 128 if nu == 2 else sz0
                            wid = 2 * QB if nu == 2 else QB
                            nc.vector.tensor_tensor(ex[0:rows, 0:wid],
                                                    e1[0:rows, 0:wid].bitcast(BF16),
                                                    e2[0:rows, 0:wid].bitcast(BF16), ADD)
                            exs[jp] = ex
                    if jp >= 1:
                        jj = jp - 1
                        ex = exs[jj]
                        j0, j1 = 2 * jj, 2 * jj + 1
                        blA = b * 25 + j0
                        blB = b * 25 + j1
                        # 4 concurrent PE tiles -> 4 banks:
                        # A-top->(0,0)->W[0:49], A-bot->(64,64)->X[64:113],
                        # B-top->(0,64)->Y[64:113], B-bot->(64,0)->Z[0:49]
                        nc.tensor.matmul(psW[0:VS, :], g_vaug[0:64, blA * VS:blA * VS + VS],
                                         ex[0:64, 0:QB], start=(jj == 0), stop=(jj == 12))
                        if j0 < 24:
                            nc.tensor.matmul(psX[64:64 + VS, :],
                                             g_vaug[64:128, blA * VS:blA * VS + VS],
                                             ex[64:128, 0:QB], start=(jj == 0),
                                             stop=(jj == 11))
                        if j1 < 25:
                            nc.tensor.matmul(psY[64:64 + VS, :],
                                             g_vaug[0:64, blB * VS:blB * VS + VS],
                                             ex[0:64, QB:2 * QB], start=(jj == 0),
                                             stop=(jj == 11))
                            nc.tensor.matmul(psZ[0:VS, :],
                                             g_vaug[64:128, blB * VS:blB * VS + VS],
                                             ex[64:128, QB:2 * QB], start=(jj == 0),
                                             stop=(jj == 11))
                # evacuate: W+Z -> outT lo half, X+Y -> hi half (the K=112
                # projection sums the halves; dens land in rows 48 and 112)
                t1 = work.tile([128, QB], F32, name="t1", tag="t1", bufs=2)
                nc.scalar.copy(t1[0:VS, :], psZ[0:VS, :])
                nc.scalar.copy(t1[64:64 + VS, :], psY[64:64 + VS, :])
                nc.vector.tensor_tensor(g_outT[0:VS, q0:q0 + QB],
                                        psW[0:VS, :], t1[0:VS, :], ADD)
                nc.vector.tensor_tensor(g_outT[64:64 + VS, q0:q0 + QB],
                                        psX[64:64 + VS, :], t1[64:64 + VS, :], ADD)
                nc.sync.dma_start(dout["g_den"][0:1, q0:q0 + QB],
                                  g_outT[48:49, q0:q0 + QB])
                nc.sync.dma_start(dout["g_den"][1:2, q0:q0 + QB],
                                  g_outT[112:113, q0:q0 + QB])

        # ---- final out-projection phase (deep ring, K=112 with [wp; 0; wp]) ----
        for blk in range(56):
            t0 = blk * 112
            for outT, wp, dst, eng in ((g_outT, gwp, dout["g_out"], "v"),
                                       (l_outT, lwp, dout["l_out"], "s")):
                pp = psum.tile([112, C], F32, name="pp", tag="psO", bufs=4)
                nc.tensor.matmul(pp[:, :], outT[0:112, t0:t0 + 112], wp[:, :],
                                 start=True, stop=True)
                sp = work.tile([112, C], BF16, name="sproj", tag="sproj", bufs=6)
                if eng == "v":
                    nc.vector.tensor_copy(sp[:, :], pp[:, :])
                else:
                    nc.scalar.copy(sp[:, :], pp[:, :])
                nc.sync.dma_start(dst[t0:t0 + 112, :], sp[:, :])


def _host_prep(x, g_qkv_w, g_proj_w, l_qkv_w, l_proj_w):
    bf = ml_dtypes.bfloat16
    xf = np.asarray(x, np.float32).reshape(NT, C)
    xT = np.ascontiguousarray(xf.T).astype(bf)
    x4 = np.asarray(x, np.float32).reshape(B, 56, 56, C)
    win = x4.reshape(B, 8, WS, 8, WS, C).transpose(0, 1, 3, 5, 2, 4)
    win = win.reshape(B, 8, 8, WS, WS, C).transpose(0, 1, 2, 4, 3, 5).reshape(NT, C)
    winT = np.ascontiguousarray(win.T).astype(bf)

    in_maps = []
    for h in range(8):
        m = {"xT": xT, "winT": winT}
        for pre, qkv_w, proj_w in (("g", g_qkv_w, g_proj_w), ("l", l_qkv_w, l_proj_w)):
            qw = np.asarray(qkv_w[:, h * HD:(h + 1) * HD], np.float32)
            kw = np.asarray(qkv_w[:, C + h * HD:C + (h + 1) * HD], np.float32)
            vw = np.asarray(qkv_w[:, 2 * C + h * HD:2 * C + (h + 1) * HD], np.float32)
            wqk = np.zeros((C, 112), np.float32)
            wqk[:, 0:48] = qw
            wqk[:, 64:112] = kw
            m[pre + "wqk"] = wqk.astype(bf)
            m[pre + "wv"] = np.ascontiguousarray(vw).astype(bf)
            wph = np.asarray(proj_w, np.float32)[h * HD:(h + 1) * HD, :]
            wp2 = np.zeros((112, C), np.float32)
            wp2[0:48] = wph
            wp2[64:112] = wph
            m[pre + "wp"] = wp2.astype(bf)
        in_maps.append(m)
    return in_maps


_NC_CACHE = None


def kernel(x, g_qkv_w, g_proj_w, g_proj_b, l_qkv_w, l_proj_w, l_proj_b):
    global _NC_CACHE
    if _NC_CACHE is None:
        _NC_CACHE = build_program()
    nc = _NC_CACHE

    in_maps = _host_prep(x, g_qkv_w, g_proj_w, l_qkv_w, l_proj_w)
    res = bass_utils.run_bass_kernel_spmd(nc, in_maps, core_ids=list(range(8)))

    acc = np.zeros((NT, C), np.float32)
    l_acc = np.zeros((NT, C), np.float32)
    for h in range(8):
        r = res.results[h]
        gden = np.asarray(r["g_den"], np.float32).sum(0).reshape(NT, 1)
        lden = np.asarray(r["l_den"], np.float32).sum(0).reshape(NT, 1)
        acc += np.asarray(r["g_out"], np.float32) / gden
        l_acc += np.asarray(r["l_out"], np.float32) / lden
    l_tok = l_acc.reshape(B, 8, 8, WS, WS, C).transpose(0, 1, 3, 2, 4, 5).reshape(NT, C)
    out = acc + l_tok + np.asarray(g_proj_b, np.float32) + np.asarray(l_proj_b, np.float32)
    return out.reshape(B, N, C).astype(np.float32)


# revision 15
# speedup vs baseline: 1.1081x; 1.0379x over previous
"""Trainium2 Bass kernel for LGAttention (global MHA + windowed local MHA).

Sharding: one attention head per NeuronCore (8 heads, 8 cores), SPMD.
v3: global S as a concurrent row-tile pair (rows 0:48 / 64:112 of the PE
array, M=128 k-tokens each, two k-blocks in flight); PV split into k-halves
accumulating into two banks (P=top halves, Q=bottom halves); exp alternates
between the scalar engine (exact) and the vector engine (two-term
phase-shifted Schraudolph bf16 bit-trick, max rel err 1.2%); local window
attention uses 4-way 64x64 array tiling with exact exp; out-projection runs
as a final deep-pipelined phase with K=112 against [wp; 0; wp] so the local
branch's parity halves sum inside the matmul. Host divides by softmax
denominators, un-permutes windows, sums the 8 per-head partials, adds biases.
"""

import sys

sys.path.insert(0, "/opt/trn_rl_repo")

import numpy as np
import ml_dtypes

import concourse.bass as bass
import concourse.mybir as mybir
import concourse.tile as tile
from concourse import bacc, bass_utils

BF16 = mybir.dt.bfloat16
F32 = mybir.dt.float32
I16 = mybir.dt.int16

B, N, C = 2, 3136, 384
H, HD, WS = 8, 48, 7
NT = B * N            # 6272 tokens total
WT = WS * WS          # 49 tokens per window
QB = 448              # q-tile (free dim) for global attention
VS = 49               # v_aug column stride: 48 v + 1 ones (denominator row)
SCALE = float(HD) ** -0.5
# two-term Schraudolph bf16 exp: exp(x) ~ bitcast(i16(A*x+B1)) + bitcast(i16(A*x+B2))
SCH_A = 128.0 * 1.4426950408889634
SCH_B1 = 16149.25
SCH_B2 = 16088.75
EXP = mybir.ActivationFunctionType.Exp
MUL = mybir.AluOpType.mult
ADD = mybir.AluOpType.add
ACT_JP = (0, 1, 2, 3, 4, 6, 7, 9, 10, 12)   # kb-pairs with exact exp (scalar engine)
NCHUNK = 4                          # input DMA chunking


def build_program():
    nc = bacc.Bacc(
        "TRN2",
        target_bir_lowering=False,
        debug=False,
        enable_asserts=False,
        num_devices=8,
    )

    din = {}
    for name, shape in [
        ("xT", (C, NT)), ("winT", (C, NT)),
        ("gwqk", (C, 112)), ("gwv", (C, HD)), ("gwp", (112, C)),
        ("lwqk", (C, 112)), ("lwv", (C, HD)), ("lwp", (112, C)),
    ]:
        din[name] = nc.dram_tensor(name, list(shape), BF16, kind="ExternalInput").ap()

    dout = {}
    for name, shape, dt in [
        ("g_out", (NT, C), BF16), ("l_out", (NT, C), BF16),
        ("g_den", (2, NT), BF16), ("l_den", (2, NT), BF16),
    ]:
        dout[name] = nc.dram_tensor(name, list(shape), dt, kind="ExternalOutput").ap()

    with tile.TileContext(nc) as tc:
        _emit(tc, nc, din, dout)

    nc.compile()
    return nc


def _emit(tc, nc, din, dout):
    from contextlib import ExitStack

    ctx = ExitStack()
    with ctx:
        persist = ctx.enter_context(tc.tile_pool(name="persist", bufs=1))
        psum = ctx.enter_context(tc.tile_pool(name="psum", bufs=2, space="PSUM"))
        work = ctx.enter_context(tc.tile_pool(name="work", bufs=3))

        # ---- weights first (small), then chunked bulk loads ----
        gwqk = persist.tile([128, 3 * 112], BF16, name="gwqk")
        lwqk = persist.tile([128, 3 * 112], BF16, name="lwqk")
        gwv = persist.tile([128, 3 * 48], BF16, name="gwv")
        lwv = persist.tile([128, 3 * 48], BF16, name="lwv")
        for c in range(3):
            nc.sync.dma_start(gwqk[:, c * 112:(c + 1) * 112], din["gwqk"][c * 128:(c + 1) * 128, :])
            nc.sync.dma_start(lwqk[:, c * 112:(c + 1) * 112], din["lwqk"][c * 128:(c + 1) * 128, :])
            nc.sync.dma_start(gwv[:, c * 48:(c + 1) * 48], din["gwv"][c * 128:(c + 1) * 128, :])
            nc.sync.dma_start(lwv[:, c * 48:(c + 1) * 48], din["lwv"][c * 128:(c + 1) * 128, :])
        gwp = persist.tile([112, C], BF16, name="gwp")
        lwp = persist.tile([112, C], BF16, name="lwp")
        nc.sync.dma_start(gwp[:, :], din["gwp"][:, :])
        nc.sync.dma_start(lwp[:, :], din["lwp"][:, :])

        xt = [persist.tile([128, NT], BF16, name=f"xt{c}") for c in range(3)]
        wt = [persist.tile([128, NT], BF16, name=f"wt{c}") for c in range(3)]
        CH = NT // NCHUNK
        for ch in range(NCHUNK):
            for c in range(3):
                nc.sync.dma_start(xt[c][:, ch * CH:(ch + 1) * CH],
                                  din["xT"][c * 128:(c + 1) * 128, ch * CH:(ch + 1) * CH])
        for ch in range(NCHUNK):
            for c in range(3):
                nc.sync.dma_start(wt[c][:, ch * CH:(ch + 1) * CH],
                                  din["winT"][c * 128:(c + 1) * 128, ch * CH:(ch + 1) * CH])

        # ---- persistent intermediates (q/k lo rows 0:48, hi rows 64:112) ----
        g_qT = persist.tile([128, NT], BF16, name="g_qT")
        g_kT = persist.tile([128, NT], BF16, name="g_kT")
        l_qT = persist.tile([128, NT], BF16, name="l_qT")
        l_kT = persist.tile([128, NT], BF16, name="l_kT")
        g_vaug = persist.tile([128, 50 * VS], BF16, name="g_vaug")
        l_vaug = persist.tile([128, 64 * VS], BF16, name="l_vaug")
        # out^T: global rows 0:48 (+den 48); local parity halves rows 0:48/64:112
        # (+dens 48/112). Out-projection contracts K=112 against [wp; 0; wp].
        g_outT = persist.tile([128, NT], BF16, name="g_outT")
        l_outT = persist.tile([128, NT], BF16, name="l_outT")
        nc.vector.memset(l_outT[:, :], 0.0)
        nc.vector.memset(g_outT[:, :], 0.0)

        # ones columns (softmax denominator comes out of the PV matmul, row 48)
        nc.vector.memset(g_vaug[:, :].rearrange("p (b k) -> p b k", k=VS)[:, :, 48:VS], 1.0)
        nc.vector.memset(l_vaug[:, :].rearrange("p (b k) -> p b k", k=VS)[:, :, 48:VS], 1.0)

        # ---- projections (x-consumers first, then win-consumers) ----
        def qk_proj(src, qT, kT, wqk):
            for qb in range(14):
                t0 = qb * QB
                ps = psum.tile([112, QB], F32, name="pqk", tag="psO", bufs=4)
                for c in range(3):
                    nc.tensor.matmul(ps[:, :], wqk[:, c * 112:(c + 1) * 112],
                                     src[c][:, t0:t0 + QB], start=(c == 0), stop=(c == 2))
                nc.scalar.copy(qT[0:48, t0:t0 + QB], ps[0:48, :])
                nc.vector.tensor_copy(kT[0:48, t0:t0 + QB], ps[64:112, :])
            for t in (qT, kT):
                nc.sync.dma_start(t[64:112, :], t[0:48, :])

        qk_proj(xt, g_qT, g_kT, gwqk)
        # global v projection (token-major), fills g_vaug cols 0:48
        for b in range(2):
            for j in range(25):
                sz = 128 if j < 24 else 64
                t0 = b * N + j * 128
                bl = b * 25 + j
                ps = psum.tile([128, HD], F32, name="pv", tag="psO", bufs=4)
                for c in range(3):
                    nc.tensor.matmul(ps[0:sz, :], xt[c][:, t0:t0 + sz],
                                     gwv[:, c * 48:(c + 1) * 48], start=(c == 0), stop=(c == 2))
                nc.vector.tensor_copy(g_vaug[0:sz, bl * VS:bl * VS + 48], ps[0:sz, :])

        qk_proj(wt, l_qT, l_kT, lwqk)
        # local v projection: col-tiled window pairs, 8 windows per psum
        for g8 in range(16):
            ps = psum.tile([128, 8 * 48], F32, name="pvl", tag="psO", bufs=4)
            for wi in range(8):
                w = g8 * 8 + wi
                t0 = w * WT
                r0 = 0 if w % 2 == 0 else 64
                for c in range(3):
                    nc.tensor.matmul(ps[r0:r0 + WT, wi * 48:(wi + 1) * 48],
                                     wt[c][:, t0:t0 + WT],
                                     lwv[:, c * 48:(c + 1) * 48], start=(c == 0), stop=(c == 2))
            dst_lo = l_vaug[0:WT, :].rearrange("p (w k) -> p w k", k=VS)[:, g8 * 4:(g8 + 1) * 4, 0:48]
            src_lo = ps[0:WT, :].rearrange("p (w k) -> p w k", k=48)[:, 0:8:2, :]
            dst_hi = l_vaug[64:64 + WT, :].rearrange("p (w k) -> p w k", k=VS)[:, g8 * 4:(g8 + 1) * 4, 0:48]
            src_hi = ps[64:64 + WT, :].rearrange("p (w k) -> p w k", k=48)[:, 1:8:2, :]
            nc.scalar.copy(dst_lo, src_lo)
            nc.vector.tensor_copy(dst_hi, src_hi)

        # ---- local attention: 8 iterations of 16 windows (2 S banks, exact exp) ----
        for it in range(8):
            psA = psum.tile([128, 8 * WT], F32, name="pSlA", tag="pS", bufs=2)
            psB = psum.tile([128, 8 * WT], F32, name="pSlB", tag="pS", bufs=2)
            for wi in range(16):
                w = it * 16 + wi
                t0 = w * WT
                bank = psA if wi < 8 else psB
                r0, r1 = (0, 48) if wi < 8 else (64, 112)
                orow = 0 if wi % 2 == 0 else 64
                col = (wi % 8) * WT
                nc.tensor.matmul(bank[orow:orow + WT, col:col + WT],
                                 l_kT[r0:r1, t0:t0 + WT], l_qT[r0:r1, t0:t0 + WT],
                                 start=True, stop=True)
            exA = work.tile([128, 8 * WT], BF16, name="expSlA", tag="exl", bufs=3)
            exB = work.tile([128, 8 * WT], BF16, name="expSlB", tag="exl2", bufs=3)

            def _wv(t, r0, par):
                return t[r0:r0 + VS, :].rearrange("p (w k) -> p w k", k=WT)[:, par:8:2, :]

            nc.scalar.activation(_wv(exA, 0, 0), _wv(psA, 0, 0), EXP, scale=SCALE)
            nc.scalar.activation(_wv(exA, 64, 1), _wv(psA, 64, 1), EXP, scale=SCALE)
            nc.scalar.activation(_wv(exB, 0, 0), _wv(psB, 0, 0), EXP, scale=SCALE)
            nc.scalar.activation(_wv(exB, 64, 1), _wv(psB, 64, 1), EXP, scale=SCALE)
            poA = psum.tile([128, 8 * WT], F32, name="poutlA", tag="psO", bufs=4)
            poB = psum.tile([128, 8 * WT], F32, name="poutlB", tag="psO", bufs=4)
            for wi in range(16):
                w = it * 16 + wi
                po = poA if wi < 8 else poB
                ex = exA if wi < 8 else exB
                col = (wi % 8) * WT
                vrow = 0 if w % 2 == 0 else 64
                nc.tensor.matmul(po[vrow:vrow + VS, col:col + WT],
                                 l_vaug[vrow:vrow + WT, (w // 2) * VS:(w // 2) * VS + VS],
                                 ex[vrow:vrow + WT, col:col + WT],
                                 start=True, stop=True)
            # evacuate per parity half (even windows rows 0:49, odd rows 64:113)
            w0 = it * 16
            for po, base in ((poA, 0), (poB, 8)):
                wb = w0 + base
                for par, vrow, eng in ((0, 0, "s"), (1, 64, "v")):
                    src_o = po[vrow:vrow + VS, :].rearrange("p (w k) -> p w k", k=WT)[:, par:8:2, :]
                    dst_o = l_outT[vrow:vrow + VS, wb * WT:(wb + 8) * WT].rearrange(
                        "p (w k) -> p w k", k=WT)[:, par:8:2, :]
                    if eng == "s":
                        nc.scalar.copy(dst_o, src_o)
                    else:
                        nc.vector.tensor_copy(dst_o, src_o)
            nc.sync.dma_start(dout["l_den"][0:1, w0 * WT:(w0 + 16) * WT],
                              l_outT[48:49, w0 * WT:(w0 + 16) * WT])
            nc.sync.dma_start(dout["l_den"][1:2, w0 * WT:(w0 + 16) * WT],
                              l_outT[112:113, w0 * WT:(w0 + 16) * WT])

        # ---- global attention: kb pair per iteration as a concurrent row-tile
        # pair (M=128); PV accumulates k-top halves into P and k-bottom halves
        # into Q; exp alternates scalar/vector engines; PV runs one kb-pair
        # behind S so the PE never waits on exp ----
        for b in range(2):
            for s in range(7):
                q0 = b * N + s * QB
                psW = psum.tile([128, QB], F32, name="psW", tag="psO", bufs=4)
                psX = psum.tile([128, QB], F32, name="psX", tag="psO", bufs=4)
                psY = psum.tile([128, QB], F32, name="psY", tag="psO", bufs=4)
                psZ = psum.tile([128, QB], F32, name="psZ", tag="psO", bufs=4)
                exs = [None] * 13
                for jp in range(15):
                    if jp < 13:
                        j0, j1 = 2 * jp, 2 * jp + 1
                        k0 = b * N + j0 * 128
                        k1 = b * N + j1 * 128
                        sz0 = 128 if j0 < 24 else 64
                        have1 = j1 < 25
                        # S(j0) at bank 0 cols 0:448, S(j1) at bank 1 cols 512:960
                        ps2 = psum.tile([128, 1024], F32, name="pS2", tag="pS", bufs=2)
                        nc.tensor.matmul(ps2[0:sz0, 0:QB], g_kT[0:48, k0:k0 + sz0],
                                         g_qT[0:48, q0:q0 + QB], start=True, stop=True)
                        if have1:
                            nc.tensor.matmul(ps2[0:128, 512:512 + QB],
                                             g_kT[64:112, k1:k1 + 128],
                                             g_qT[64:112, q0:q0 + QB], start=True, stop=True)
                        nu = 2 if have1 else 1
                        ps_v = (ps2[0:128, :].rearrange("p (u k) -> p u k", k=512)[:, 0:nu, 0:QB]
                                if nu == 2 else ps2[0:sz0, 0:QB])
                        if jp in ACT_JP:
                            ex = work.tile([128, 2 * QB], BF16, name="expA", tag="exA", bufs=3)
                            ex_v = (ex[0:128, :].rearrange("p (u k) -> p u k", k=QB)[:, 0:nu, :]
                                    if nu == 2 else ex[0:sz0, 0:QB])
                            nc.scalar.activation(ex_v, ps_v, EXP, scale=SCALE)
                            exs[jp] = ex
                        else:
                            e1 = work.tile([128, 2 * QB], I16, name="exi1", tag="exi1", bufs=3)
                            e2 = work.tile([128, 2 * QB], I16, name="exi2", tag="exi2", bufs=3)
                            ex = work.tile([128, 2 * QB], BF16, name="expB", tag="exB", bufs=3)
                            for e, bconst in ((e1, SCH_B1), (e2, SCH_B2)):
                                e_v = (e[0:128, :].rearrange("p (u k) -> p u k", k=QB)[:, 0:nu, :]
                                       if nu == 2 else e[0:sz0, 0:QB])
                                nc.vector.tensor_scalar(e_v, ps_v,
                                                        SCALE * SCH_A, bconst, MUL, ADD)
                            rows = 128 if nu == 2 else sz0
                            wid = 2 * QB if nu == 2 else QB
                            nc.vector.tensor_tensor(ex[0:rows, 0:wid],
                                                    e1[0:rows, 0:wid].bitcast(BF16),
                                                    e2[0:rows, 0:wid].bitcast(BF16), ADD)
                            exs[jp] = ex
                    if jp >= 2:
                        jj = jp - 2
                        ex = exs[jj]
                        j0, j1 = 2 * jj, 2 * jj + 1
                        blA = b * 25 + j0
                        blB = b * 25 + j1
                        # 4 concurrent PE tiles -> 4 banks:
                        # A-top->(0,0)->W[0:49], A-bot->(64,64)->X[64:113],
                        # B-top->(0,64)->Y[64:113], B-bot->(64,0)->Z[0:49]
                        nc.tensor.matmul(psW[0:VS, :], g_vaug[0:64, blA * VS:blA * VS + VS],
                                         ex[0:64, 0:QB], start=(jj == 0), stop=(jj == 12))
                        if j0 < 24:
                            nc.tensor.matmul(psX[64:64 + VS, :],
                                             g_vaug[64:128, blA * VS:blA * VS + VS],
                                             ex[64:128, 0:QB], start=(jj == 0),
                                             stop=(jj == 11))
                        if j1 < 25:
                            nc.tensor.matmul(psY[64:64 + VS, :],
                                             g_vaug[0:64, blB * VS:blB * VS + VS],
                                             ex[0:64, QB:2 * QB], start=(jj == 0),
                                             stop=(jj == 11))
                            nc.tensor.matmul(psZ[0:VS, :],
                                             g_vaug[64:128, blB * VS:blB * VS + VS],
                                             ex[64:128, QB:2 * QB], start=(jj == 0),
                                             stop=(jj == 11))
                # evacuate: W+Z -> outT lo half, X+Y -> hi half (the K=112
                # projection sums the halves; dens land in rows 48 and 112)
                t1 = work.tile([128, QB], F32, name="t1", tag="t1", bufs=2)
                nc.scalar.copy(t1[0:VS, :], psZ[0:VS, :])
                nc.scalar.copy(t1[64:64 + VS, :], psY[64:64 + VS, :])
                nc.vector.tensor_tensor(g_outT[0:VS, q0:q0 + QB],
                                        psW[0:VS, :], t1[0:VS, :], ADD)
                nc.vector.tensor_tensor(g_outT[64:64 + VS, q0:q0 + QB],
                                        psX[64:64 + VS, :], t1[64:64 + VS, :], ADD)
                nc.sync.dma_start(dout["g_den"][0:1, q0:q0 + QB],
                                  g_outT[48:49, q0:q0 + QB])
                nc.sync.dma_start(dout["g_den"][1:2, q0:q0 + QB],
                                  g_outT[112:113, q0:q0 + QB])

        # ---- final out-projection phase (deep ring, K=112 with [wp; 0; wp]) ----
        for blk in range(56):
            t0 = blk * 112
            for outT, wp, dst, eng in ((g_outT, gwp, dout["g_out"], "v"),
                                       (l_outT, lwp, dout["l_out"], "s")):
                pp = psum.tile([112, C], F32, name="pp", tag="psO", bufs=4)
                nc.tensor.matmul(pp[:, :], outT[0:112, t0:t0 + 112], wp[:, :],
                                 start=True, stop=True)
                sp = work.tile([112, C], BF16, name="sproj", tag="sproj", bufs=6)
                if eng == "v":
                    nc.vector.tensor_copy(sp[:, :], pp[:, :])
                else:
                    nc.scalar.copy(sp[:, :], pp[:, :])
                nc.sync.dma_start(dst[t0:t0 + 112, :], sp[:, :])


def _host_prep(x, g_qkv_w, g_proj_w, l_qkv_w, l_proj_w):
    bf = ml_dtypes.bfloat16
    xf = np.asarray(x, np.float32).reshape(NT, C)
    xT = np.ascontiguousarray(xf.T).astype(bf)
    x4 = np.asarray(x, np.float32).reshape(B, 56, 56, C)
    win = x4.reshape(B, 8, WS, 8, WS, C).transpose(0, 1, 3, 5, 2, 4)
    win = win.reshape(B, 8, 8, WS, WS, C).transpose(0, 1, 2, 4, 3, 5).reshape(NT, C)
    winT = np.ascontiguousarray(win.T).astype(bf)

    in_maps = []
    for h in range(8):
        m = {"xT": xT, "winT": winT}
        for pre, qkv_w, proj_w in (("g", g_qkv_w, g_proj_w), ("l", l_qkv_w, l_proj_w)):
            qw = np.asarray(qkv_w[:, h * HD:(h + 1) * HD], np.float32)
            kw = np.asarray(qkv_w[:, C + h * HD:C + (h + 1) * HD], np.float32)
            vw = np.asarray(qkv_w[:, 2 * C + h * HD:2 * C + (h + 1) * HD], np.float32)
            wqk = np.zeros((C, 112), np.float32)
            wqk[:, 0:48] = qw
            wqk[:, 64:112] = kw
            m[pre + "wqk"] = wqk.astype(bf)
            m[pre + "wv"] = np.ascontiguousarray(vw).astype(bf)
            wph = np.asarray(proj_w, np.float32)[h * HD:(h + 1) * HD, :]
            wp2 = np.zeros((112, C), np.float32)
            wp2[0:48] = wph
            wp2[64:112] = wph
            m[pre + "wp"] = wp2.astype(bf)
        in_maps.append(m)
    return in_maps


_NC_CACHE = None


def kernel(x, g_qkv_w, g_proj_w, g_proj_b, l_qkv_w, l_proj_w, l_proj_b):
    global _NC_CACHE
    if _NC_CACHE is None:
        _NC_CACHE = build_program()
    nc = _NC_CACHE

    in_maps = _host_prep(x, g_qkv_w, g_proj_w, l_qkv_w, l_proj_w)
    res = bass_utils.run_bass_kernel_spmd(nc, in_maps, core_ids=list(range(8)))

    acc = np.zeros((NT, C), np.float32)
    l_acc = np.zeros((NT, C), np.float32)
    for h in range(8):
        r = res.results[h]
        gden = np.asarray(r["g_den"], np.float32).sum(0).reshape(NT, 1)
        lden = np.asarray(r["l_den"], np.float32).sum(0).reshape(NT, 1)
        acc += np.asarray(r["g_out"], np.float32) / gden
        l_acc += np.asarray(r["l_out"], np.float32) / lden
    l_tok = l_acc.reshape(B, 8, 8, WS, WS, C).transpose(0, 1, 3, 2, 4, 5).reshape(NT, C)
    out = acc + l_tok + np.asarray(g_proj_b, np.float32) + np.asarray(l_proj_b, np.float32)
    return out.reshape(B, N, C).astype(np.float32)
